# revision 5
# baseline (speedup 1.0000x reference)
"""Trainium2 Bass kernel for nn_BCDReverseTransform (segment_reduce).

Computes y[n] = sum_j 2^j * (sign(x[n,j])+1)/2  for x [4M, 16] f32.

Identity used on-device:  y = 0.5 * (sum_j 2^j * sign(x_j)) + 32767.5
 - ACT engine: s = Sign(x)  (f32 -> bf16, handles x==+-0 -> 0 exactly)
 - DVE: m = s * wrep        (bf16 2x mode; products +-2^j exact in bf16)
 - DVE: segmented reduce groups of 16 (f32 accumulate, exact)
 - ACT engine: y = 0.5*r + 32767.5 (exact: all values are multiples of 0.5
   below 2^17, representable in f32)

Sharding: data-parallel on the row axis across 8 cores; each core gets
500,000 rows, padded to 500,096 = 128*3907 so rows split evenly over the
128 SBUF partitions. Row-major layout keeps every DMA fully contiguous
per partition.
"""

import numpy as np
import ml_dtypes

N_CORES = 8
D = 16
ROWS_TOTAL = 4_000_000
ROWS_PER_CORE = ROWS_TOTAL // N_CORES  # 500_000
ROWS_PAD_PER_CORE = 500_096  # = 128 * 3907
RPP = ROWS_PAD_PER_CORE // 128  # 3907 rows per partition
TILE_ROWS = 256  # rows per partition per tile -> F = 4096 free elems


def tile_splits(rpp=RPP, tile_rows=TILE_ROWS):
    out = []
    r = rpp
    while r > 0:
        t = min(tile_rows, r)
        out.append(t)
        r -= t
    return out


def build_nc(rows_pad=ROWS_PAD_PER_CORE, tile_rows=TILE_ROWS, reps=1):
    """Build + compile the single-core Bass program (SPMD across 8 cores).

    reps > 1 repeats the whole tile loop (same inputs/outputs) inside one
    program — used only by the dev harness to measure steady-state HW time
    as a slope between two rep counts, canceling host/RPC overhead.
    """
    from contextlib import ExitStack

    import concourse.bacc as bacc
    import concourse.mybir as mybir
    import concourse.tile as tile

    f32 = mybir.dt.float32
    bf16 = mybir.dt.bfloat16
    rpp = rows_pad // 128
    assert rows_pad % 128 == 0
    splits = tile_splits(rpp, tile_rows)
    fmax = tile_rows * D

    nc = bacc.Bacc("TRN2", target_bir_lowering=False, debug=False)
    x = nc.dram_tensor("x", [rows_pad * D], f32, kind="ExternalInput").ap()
    wrep = nc.dram_tensor("wrep", [128, fmax], bf16, kind="ExternalInput").ap()
    y = nc.dram_tensor("y", [rows_pad], f32, kind="ExternalOutput").ap()

    with tile.TileContext(nc) as tc, ExitStack() as ctx:
        wpool = ctx.enter_context(tc.tile_pool(name="wrep", bufs=1))
        wt = wpool.tile([128, fmax], bf16)
        nc.sync.dma_start(out=wt[:], in_=wrep[:, :])

        xpool = ctx.enter_context(tc.tile_pool(name="xin", bufs=3))
        mpool = ctx.enter_context(tc.tile_pool(name="mid", bufs=3))
        opool = ctx.enter_context(tc.tile_pool(name="out", bufs=3))

        for _rep in range(reps):
            off = 0
            yoff = 0
            for rt in splits:
                F = rt * D
                xt = xpool.tile([128, F], f32, tag="x")
                nc.sync.dma_start(
                    out=xt[:],
                    in_=x[off : off + 128 * F].rearrange("(p f) -> p f", p=128),
                )
                st = mpool.tile([128, F], bf16, tag="s")
                nc.scalar.activation(
                    st[:], xt[:], mybir.ActivationFunctionType.Sign
                )
                mt = mpool.tile([128, F], bf16, tag="m")
                nc.vector.tensor_tensor(
                    mt[:], st[:], wt[:, :F], op=mybir.AluOpType.mult
                )
                red = opool.tile([128, rt], f32, tag="r")
                nc.vector.tensor_reduce(
                    red[:],
                    mt[:].rearrange("p (g j) -> p g j", j=D),
                    axis=mybir.AxisListType.X,
                    op=mybir.AluOpType.add,
                )
                yt = opool.tile([128, rt], f32, tag="y")
                nc.scalar.activation(
                    yt[:],
                    red[:],
                    mybir.ActivationFunctionType.Copy,
                    bias=32767.5,
                    scale=0.5,
                )
                nc.sync.dma_start(
                    out=y[yoff : yoff + 128 * rt].rearrange("(p f) -> p f", p=128),
                    in_=yt[:],
                )
                off += 128 * F
                yoff += 128 * rt

    nc.compile()
    return nc


def make_wrep(tile_rows=TILE_ROWS):
    w16 = np.exp2(np.arange(D, dtype=np.float32))
    row = np.tile(w16, tile_rows).astype(ml_dtypes.bfloat16)
    return np.broadcast_to(row, (128, tile_rows * D)).copy()


_CACHE = {}


def kernel(x):
    x = np.ascontiguousarray(np.asarray(x), dtype=np.float32)
    assert x.shape == (ROWS_TOTAL, D)

    if "nc" not in _CACHE:
        _CACHE["nc"] = build_nc()
    nc = _CACHE["nc"]

    wrep = make_wrep()
    pad = np.zeros((ROWS_PAD_PER_CORE - ROWS_PER_CORE, D), np.float32)
    in_maps = []
    for c in range(N_CORES):
        xs = x[c * ROWS_PER_CORE : (c + 1) * ROWS_PER_CORE]
        xpad = np.concatenate([xs, pad], axis=0).reshape(-1)
        in_maps.append({"x": xpad, "wrep": wrep})

    from concourse.bass_utils import run_bass_kernel_spmd

    res = run_bass_kernel_spmd(nc, in_maps, list(range(N_CORES)))
    y = np.concatenate([r["y"][:ROWS_PER_CORE] for r in res.results])
    return y


# revision 9
# speedup vs baseline: 429.9615x; 429.9615x over previous
"""Trainium2 Bass kernel for nn_BCDReverseTransform (segment_reduce).

Computes y[n] = sum_j 2^j * (sign(x[n,j])+1)/2  for x [4M, 16] f32.

Identity used on-device:  y = 0.5 * z + 32767.5,  z = sum_j 2^j*sign(x_j)
 - ACT:  s = Sign(x), f32 -> bf16 (handles +-0 -> 0 exactly; 1-ULP func)
 - DVE:  4-level scalar_tensor_tensor ladder, weights folded into the
   per-level uniform scalars (adjacent weights differ by a constant
   ratio), no weight tile and no tensor_reduce needed:
       t1 = 2*s_odd   + s_even    (|t1| <= 3,   bf16 exact)
       t2 = 4*t1_odd  + t1_even   (|t2| <= 15,  bf16 exact)
       t3 = 16*t2_odd + t2_even   (|t3| <= 255, bf16 exact)
       z  = 256*t3_odd+ t3_even   (|z| <= 65535, f32 exact)
 - ACT:  y = 0.5*z + 32767.5 (all values are multiples of 0.5 below
   2^17 -> exact in f32; result is bit-exact vs the reference math)

Sharding: data-parallel on rows across 8 cores (500,000 rows each,
padded to 500,096 = 128*3907 so rows split evenly over 128 SBUF
partitions). Row-major layout keeps every DMA contiguous per partition:
per-core HBM traffic is 32 MB in + 2 MB out, and the measured kernel
runs at ~100 us/core steady-state = the ~358 GB/s HBM-per-core limit.

Tiling: 7 tiles of 512 rows/partition (4 MB DMAs, past the DMA-size
knee) + one 323-row remainder; input pool 4 deep so the DMA stream
never waits on compute; outputs ride the same HWDGE ring (SP engine).
A dummy Sign on a [1,2] tile preloads the ACT spline table under the
first DMA.
"""

from contextlib import ExitStack

import numpy as np

N_CORES = 8
D = 16
ROWS_TOTAL = 4_000_000
ROWS_PER_CORE = ROWS_TOTAL // N_CORES  # 500_000
ROWS_PAD_PER_CORE = 500_096  # = 128 * 3907
RPP = ROWS_PAD_PER_CORE // 128  # 3907 rows per partition
TILE_ROWS = 512  # rows per partition per tile -> 4 MB input DMAs


def tile_splits(rpp=RPP, tile_rows=TILE_ROWS):
    out = []
    r = rpp
    while r > 0:
        t = min(tile_rows, r)
        out.append(t)
        r -= t
    return out


def build_nc(rows_pad=ROWS_PAD_PER_CORE, tile_rows=TILE_ROWS, reps=1, loop_n=1):
    """Build + compile the single-core Bass program (SPMD across 8 cores).

    reps/loop_n (>1) repeat the body (python-unrolled / hardware For_i) —
    used only by the dev harness for steady-state timing via slopes.
    """
    import concourse.bacc as bacc
    import concourse.mybir as mybir
    import concourse.tile as tile

    f32 = mybir.dt.float32
    bf16 = mybir.dt.bfloat16
    rpp = rows_pad // 128
    assert rows_pad % 128 == 0
    splits = tile_splits(rpp, tile_rows)

    nc = bacc.Bacc("TRN2", target_bir_lowering=False, debug=False)
    x = nc.dram_tensor("x", [rows_pad * D], f32, kind="ExternalInput").ap()
    y = nc.dram_tensor("y", [rows_pad], f32, kind="ExternalOutput").ap()

    def pairs(ap2d, n):
        return ap2d.rearrange("p (g two) -> p g two", two=2), n // 2

    with tile.TileContext(nc) as tc, ExitStack() as ctx:
        xpool = ctx.enter_context(tc.tile_pool(name="xin", bufs=4))
        mpool = ctx.enter_context(tc.tile_pool(name="mid", bufs=2))
        opool = ctx.enter_context(tc.tile_pool(name="out", bufs=2))

        # Preload the ACT Sign spline table under the first input DMA.
        wpool = ctx.enter_context(tc.tile_pool(name="warm", bufs=1))
        wtile = wpool.tile([1, 2], f32)
        nc.gpsimd.memset(wtile[:], 0.0)
        nc.scalar.activation(
            wtile[:, 1:2], wtile[:, 0:1], mybir.ActivationFunctionType.Sign
        )

        def emit_rep():
            off = 0
            yoff = 0
            for rt in splits:
                F = rt * D
                xt = xpool.tile([128, F], f32, tag="x")
                nc.sync.dma_start(
                    out=xt[:],
                    in_=x[off : off + 128 * F].rearrange("(p f) -> p f", p=128),
                )
                st = mpool.tile([128, F], bf16, tag="s")
                nc.scalar.activation(
                    st[:], xt[:], mybir.ActivationFunctionType.Sign
                )
                cur = st
                n = F
                for lvl, (mulc, odt) in enumerate(
                    ((2.0, bf16), (4.0, bf16), (16.0, bf16), (256.0, f32))
                ):
                    v, n2 = pairs(cur[:], n)
                    nxt = mpool.tile([128, n2], odt, tag=f"t{lvl}")
                    nc.vector.scalar_tensor_tensor(
                        nxt[:].rearrange("p (g b) -> p g b", b=1),
                        v[:, :, 1:2],
                        mulc,
                        v[:, :, 0:1],
                        op0=mybir.AluOpType.mult,
                        op1=mybir.AluOpType.add,
                    )
                    cur = nxt
                    n = n2
                yt = opool.tile([128, rt], f32, tag="y")
                nc.scalar.activation(
                    yt[:],
                    cur[:],
                    mybir.ActivationFunctionType.Copy,
                    bias=32767.5,
                    scale=0.5,
                )
                nc.sync.dma_start(
                    out=y[yoff : yoff + 128 * rt].rearrange("(p f) -> p f", p=128),
                    in_=yt[:],
                )
                off += 128 * F
                yoff += 128 * rt

        def emit_body():
            for _ in range(reps):
                emit_rep()

        if loop_n > 1:
            with tc.For_i(0, loop_n, 1):
                emit_body()
        else:
            emit_body()

    nc.compile()
    return nc


_CACHE = {}


def kernel(x):
    x = np.ascontiguousarray(np.asarray(x), dtype=np.float32)
    assert x.shape == (ROWS_TOTAL, D)

    if "nc" not in _CACHE:
        _CACHE["nc"] = build_nc()
    nc = _CACHE["nc"]

    pad = np.zeros((ROWS_PAD_PER_CORE - ROWS_PER_CORE, D), np.float32)
    in_maps = []
    for c in range(N_CORES):
        xs = x[c * ROWS_PER_CORE : (c + 1) * ROWS_PER_CORE]
        xpad = np.concatenate([xs, pad], axis=0).reshape(-1)
        in_maps.append({"x": xpad})

    from concourse.bass_utils import run_bass_kernel_spmd

    res = run_bass_kernel_spmd(nc, in_maps, list(range(N_CORES)))
    y = np.concatenate([r["y"][:ROWS_PER_CORE] for r in res.results])
    return y
